# revision 6
# baseline (speedup 1.0000x reference)
"""MoE (DeepSeek-style gate + 32 routed SwiGLU experts + shared expert) on 8 trn2 cores.

Strategy: data-parallel over tokens (batch dim 8 -> 1 slab per core), expert
weights replicated.  Two device launches per call:

  1. gate kernel: computes dense combine-weights cw[T, E] (softmax + grouped
     top-k routing) on device.
  2. main kernel: per expert, gathers its tokens (host builds the gather
     layout from cw - pure data movement), runs the SwiGLU expert MLP on
     device with fp32r matmuls, scales by routing weight, scatters rows into
     a slot buffer, then combines slots + shared-expert output + (b2/sb2 via
     a small cw @ [b2;sb2] matmul) into y.

All arithmetic happens on device; the host only reshapes/permutes data.
"""

import sys

sys.path.insert(0, "/opt/trn_rl_repo")

import ml_dtypes
import numpy as np

BF = ml_dtypes.bfloat16

import concourse.bacc as bacc
import concourse.mybir as mybir
import concourse.tile as tile
from concourse import bass
from concourse.bass_utils import run_bass_kernel_spmd
from concourse.masks import make_identity

NCORES = 8
DIM = 1024
INTER = 512
E = 32
TOPK = 4
GROUPS = 8
TOPK_G = 4
SINTER = 1024
P = 128
KD = DIM // P     # 8 k-tiles over dim
KI = INTER // P   # 4 k-tiles over inter
KS = SINTER // P  # 8 k-tiles over shared inter

F32 = mybir.dt.float32
F32R = mybir.dt.float32r
F16 = mybir.dt.float16
BF16 = mybir.dt.bfloat16
I32 = mybir.dt.int32
AF = mybir.ActivationFunctionType
OP = mybir.AluOpType
AX = mybir.AxisListType


def _chunks(total, size):
    out = []
    off = 0
    while off < total:
        out.append((off, min(size, total - off)))
        off += size
    return out


def build_gate(T):
    nc = bacc.Bacc("TRN2", target_bir_lowering=False)
    xT = nc.dram_tensor("xT", [DIM, T], F32, kind="ExternalInput")
    gw = nc.dram_tensor("gw", [DIM, E], F32, kind="ExternalInput")
    gb = nc.dram_tensor("gb", [1, E], F32, kind="ExternalInput")
    cw = nc.dram_tensor("cw", [T, E], F32, kind="ExternalOutput")
    ntile = T // P
    with tile.TileContext(nc) as tc:
        with tc.tile_pool(name="cst", bufs=1) as cst, \
             tc.tile_pool(name="sb", bufs=3) as sb, \
             tc.tile_pool(name="ps", bufs=3, space="PSUM") as ps:
            gwt = cst.tile([P, KD, E], F32)
            nc.sync.dma_start(out=gwt[:], in_=gw.ap().rearrange("(k p) e -> p k e", p=P))
            gbt = cst.tile([1, E], F32)
            nc.sync.dma_start(out=gbt[:], in_=gb.ap())
            onet = cst.tile([1, P], F32)
            nc.vector.memset(onet[:], 1.0)
            for t in range(ntile):
                xt = sb.tile([P, KD, P], F32, tag="xt")
                nc.sync.dma_start(
                    out=xt[:],
                    in_=xT.ap()[:, t * P:(t + 1) * P].rearrange("(k p) n -> p k n", p=P),
                )
                s = ps.tile([P, E], F32, tag="s")
                for k in range(KD):
                    nc.tensor.matmul(out=s[:], lhsT=xt[:, k, :], rhs=gwt[:, k, :],
                                     start=(k == 0), stop=False)
                nc.tensor.matmul(out=s[:], lhsT=onet[:], rhs=gbt[:], start=False, stop=True)
                # softmax over the 32 experts (free dim)
                negmax = sb.tile([P, 1], F32, tag="negmax")
                nc.vector.tensor_reduce(out=negmax[:], in_=s[:], op=OP.max, axis=AX.X,
                                        negate=True)
                et = sb.tile([P, E], F32, tag="et")
                nc.scalar.activation(et[:], s[:], AF.Exp, bias=negmax[:, 0:1], scale=1.0)
                ssum = sb.tile([P, 1], F32, tag="ssum")
                nc.vector.reduce_sum(out=ssum[:], in_=et[:], axis=AX.X)
                rsum = sb.tile([P, 1], F32, tag="rsum")
                nc.vector.reciprocal(rsum[:], ssum[:])
                sc = sb.tile([P, E], F32, tag="sc")
                nc.vector.tensor_scalar_mul(sc[:], et[:], rsum[:, 0:1])
                # group scores: sum of top-2 scores within each group of 4.
                # top2sum(a,b,c,d) = max(a+b, c+d, max(a,b)+max(c,d))
                g = sc[:].rearrange("p (g c) -> p g c", c=4)
                ga = sb.tile([P, GROUPS], F32, tag="ga")
                gbv = sb.tile([P, GROUPS], F32, tag="gbv")
                m1 = sb.tile([P, GROUPS], F32, tag="m1")
                gsc = sb.tile([P, GROUPS], F32, tag="gsc")
                nc.vector.tensor_add(ga[:], g[:, :, 0], g[:, :, 1])
                nc.vector.tensor_add(gbv[:], g[:, :, 2], g[:, :, 3])
                nc.vector.tensor_tensor(out=m1[:], in0=g[:, :, 0], in1=g[:, :, 1], op=OP.max)
                nc.vector.tensor_tensor(out=gsc[:], in0=g[:, :, 2], in1=g[:, :, 3], op=OP.max)
                nc.vector.tensor_add(m1[:], m1[:], gsc[:])
                nc.vector.tensor_tensor(out=ga[:], in0=ga[:], in1=gbv[:], op=OP.max)
                nc.vector.tensor_tensor(out=gsc[:], in0=ga[:], in1=m1[:], op=OP.max)
                # keep the top-4 groups
                srt = sb.tile([P, 8], F32, tag="srt")
                nc.vector.max(srt[:], gsc[:])
                keep = sb.tile([P, GROUPS], F32, tag="keep")
                nc.vector.tensor_scalar(keep[:], gsc[:], srt[:, 3:4], None, op0=OP.is_ge)
                # mask scores to kept groups, take top-4 experts
                masked = sb.tile([P, E], F32, tag="masked")
                nc.vector.tensor_tensor(
                    out=masked[:].rearrange("p (g c) -> p g c", c=4),
                    in0=g,
                    in1=keep[:].unsqueeze(2).to_broadcast([P, GROUPS, 4]),
                    op=OP.mult,
                )
                srt2 = sb.tile([P, 8], F32, tag="srt2")
                nc.vector.max(srt2[:], masked[:])
                sel = sb.tile([P, E], F32, tag="sel")
                nc.vector.tensor_scalar(sel[:], masked[:], srt2[:, 3:4], None, op0=OP.is_ge)
                cwt = sb.tile([P, E], F32, tag="cwt")
                nc.vector.tensor_mul(cwt[:], sel[:], masked[:])
                nc.sync.dma_start(out=cw.ap()[t * P:(t + 1) * P, :], in_=cwt[:])
    return nc


def build_main(T, seg_len, Lsum, rem_fast=False):
    """seg_len[e]: padded token count for expert e (same across cores).
    Lsum = sum(seg_len).  zbuf rows: slot k of token t at k*T+t, dummy
    scratch rows (padding pairs) at 4*T."""
    nc = bacc.Bacc("TRN2", target_bir_lowering=False)
    xT = nc.dram_tensor("xT", [DIM, T], BF16, kind="ExternalInput")
    xg = nc.dram_tensor("xg", [DIM, Lsum], BF16, kind="ExternalInput")
    pw = nc.dram_tensor("pw", [Lsum, 1], F32, kind="ExternalInput")
    soff = nc.dram_tensor("soff", [Lsum, 1], I32, kind="ExternalInput")
    cwT1 = nc.dram_tensor("cwT1", [E + 1, T], F32R, kind="ExternalInput")
    b2a = nc.dram_tensor("b2a", [E + 1, DIM], F32R, kind="ExternalInput")
    w1 = nc.dram_tensor("w1", [E, DIM, INTER], BF16, kind="ExternalInput")
    b1 = nc.dram_tensor("b1", [E, INTER], F32, kind="ExternalInput")
    w3 = nc.dram_tensor("w3", [E, DIM, INTER], BF16, kind="ExternalInput")
    b3 = nc.dram_tensor("b3", [E, INTER], F32, kind="ExternalInput")
    w2 = nc.dram_tensor("w2", [E, INTER, DIM], BF16, kind="ExternalInput")
    sw1 = nc.dram_tensor("sw1", [DIM, SINTER], BF16, kind="ExternalInput")
    sb1 = nc.dram_tensor("sb1", [1, SINTER], F32, kind="ExternalInput")
    sw3 = nc.dram_tensor("sw3", [DIM, SINTER], BF16, kind="ExternalInput")
    sb3 = nc.dram_tensor("sb3", [1, SINTER], F32, kind="ExternalInput")
    sw2 = nc.dram_tensor("sw2", [SINTER, DIM], BF16, kind="ExternalInput")
    y = nc.dram_tensor("y", [T, DIM], F32, kind="ExternalOutput")
    zbuf = nc.dram_tensor("zbuf", [4 * T + P, DIM], F16)

    seg_start = np.concatenate([[0], np.cumsum(seg_len)]).astype(int)

    from contextlib import ExitStack
    with tile.TileContext(nc) as tc:
        with ExitStack() as ctx:
            cst = ctx.enter_context(tc.tile_pool(name="cst", bufs=1))
            wp = ctx.enter_context(tc.tile_pool(name="wp", bufs=2))
            xp = ctx.enter_context(tc.tile_pool(name="xp", bufs=2))
            hp = ctx.enter_context(tc.tile_pool(name="hp", bufs=2))
            ep = ctx.enter_context(tc.tile_pool(name="ep", bufs=2))
            zp = ctx.enter_context(tc.tile_pool(name="zp", bufs=2))
            cp = ctx.enter_context(tc.tile_pool(name="cp", bufs=3))
            pp1 = ctx.enter_context(tc.tile_pool(name="pp1", bufs=2, space="PSUM"))
            pp2 = ctx.enter_context(tc.tile_pool(name="pp2", bufs=2, space="PSUM"))

            ident = cst.tile([P, P], F32)
            make_identity(nc, ident[:])

            def up_proj(xt, w1t, w3t, b1t, b3t, ht, m, nlen, kk):
                """ht[:, m, :nlen] = silu(w1^T x + b1) * (w3^T x + b3) for inter tile m."""
                ps1 = pp1.tile([P, 512], F32, tag="ps1")
                for k in range(kk):
                    nc.tensor.matmul(out=ps1[:, :nlen], lhsT=w1t[:, k, m * P:(m + 1) * P],
                                     rhs=xt[:, k, :nlen], start=(k == 0), stop=(k == kk - 1))
                ps3 = pp1.tile([P, 512], F32, tag="ps3")
                for k in range(kk):
                    nc.tensor.matmul(out=ps3[:, :nlen], lhsT=w3t[:, k, m * P:(m + 1) * P],
                                     rhs=xt[:, k, :nlen], start=(k == 0), stop=(k == kk - 1))
                hs = ep.tile([P, 512], F32, tag="hs")
                nc.scalar.activation(hs[:, :nlen], ps1[:, :nlen], AF.Silu,
                                     bias=b1t[:, m:m + 1], scale=1.0)
                h3 = ep.tile([P, 512], F32, tag="h3")
                nc.scalar.activation(h3[:, :nlen], ps3[:, :nlen], AF.Identity,
                                     bias=b3t[:, m:m + 1], scale=1.0)
                nc.vector.tensor_mul(ht[:, m, :nlen], hs[:, :nlen], h3[:, :nlen])

            # ---------------- phase A: routed experts ----------------
            order = [e for e in range(E) if seg_len[e] > 0]
            shared_tiles = {}

            def load_shared():
                s1a = wp.tile([P, 4, SINTER], BF16, tag="w1e")
                s1b = wp.tile([P, 4, SINTER], BF16, tag="w1e")
                nc.sync.dma_start(out=s1a[:], in_=sw1.ap()[:512, :].rearrange("(k p) i -> p k i", p=P))
                nc.sync.dma_start(out=s1b[:], in_=sw1.ap()[512:, :].rearrange("(k p) i -> p k i", p=P))
                s3a = wp.tile([P, 4, SINTER], BF16, tag="w3e")
                s3b = wp.tile([P, 4, SINTER], BF16, tag="w3e")
                nc.sync.dma_start(out=s3a[:], in_=sw3.ap()[:512, :].rearrange("(k p) i -> p k i", p=P))
                nc.sync.dma_start(out=s3b[:], in_=sw3.ap()[512:, :].rearrange("(k p) i -> p k i", p=P))
                s2a = wp.tile([P, 4, DIM], BF16, tag="w2e")
                s2b = wp.tile([P, 4, DIM], BF16, tag="w2e")
                nc.sync.dma_start(out=s2a[:], in_=sw2.ap()[:512, :].rearrange("(k p) d -> p k d", p=P))
                nc.sync.dma_start(out=s2b[:], in_=sw2.ap()[512:, :].rearrange("(k p) d -> p k d", p=P))
                shared_tiles.update(s1a=s1a, s1b=s1b, s3a=s3a, s3b=s3b, s2a=s2a, s2b=s2b)

            for ei, e in enumerate(order):
                e = int(e)
                w1t = wp.tile([P, KD, INTER], BF16, tag="w1e")
                nc.sync.dma_start(out=w1t[:], in_=w1.ap()[e].rearrange("(k p) i -> p k i", p=P))
                w3t = wp.tile([P, KD, INTER], BF16, tag="w3e")
                nc.sync.dma_start(out=w3t[:], in_=w3.ap()[e].rearrange("(k p) i -> p k i", p=P))
                w2t = wp.tile([P, KI, DIM], BF16, tag="w2e")
                nc.sync.dma_start(out=w2t[:], in_=w2.ap()[e].rearrange("(k p) d -> p k d", p=P))
                b1t = wp.tile([P, KI], F32, tag="b1e")
                nc.sync.dma_start(out=b1t[:], in_=b1.ap()[e].rearrange("(m p) -> p m", p=P))
                b3t = wp.tile([P, KI], F32, tag="b3e")
                nc.sync.dma_start(out=b3t[:], in_=b3.ap()[e].rearrange("(m p) -> p m", p=P))
                for (n0, nlen) in _chunks(int(seg_len[e]), 512):
                    g0 = seg_start[e] + n0
                    xt = xp.tile([P, KD, 512], BF16, tag="xg")
                    nc.sync.dma_start(
                        out=xt[:, :, :nlen],
                        in_=xg.ap()[:, g0:g0 + nlen].rearrange("(k p) n -> p k n", p=P),
                    )
                    nch = (nlen + P - 1) // P
                    pwt = ep.tile([P, 4], F32, tag="pwt")
                    sot = ep.tile([P, 4], I32, tag="sot")
                    for c in range(nch):
                        cl = min(P, nlen - c * P)
                        nc.sync.dma_start(out=pwt[:cl, c:c + 1], in_=pw.ap()[g0 + c * P:g0 + c * P + cl, :])
                        nc.sync.dma_start(out=sot[:cl, c:c + 1], in_=soff.ap()[g0 + c * P:g0 + c * P + cl, :])
                    ht = hp.tile([P, KS, 512], BF16, tag="ht")
                    if rem_fast and nlen <= P:
                        # token-stationary up-proj (8+8 matmuls instead of 64),
                        # then PE-transpose H -> HT.  Valid because b1/b3 == 0.
                        psH1 = pp1.tile([P, 512], F32, tag="ps1")
                        for k in range(KD):
                            nc.tensor.matmul(out=psH1[:nlen, :], lhsT=xt[:, k, :nlen],
                                             rhs=w1t[:, k, :], start=(k == 0), stop=(k == KD - 1))
                        psH3 = pp1.tile([P, 512], F32, tag="ps3")
                        for k in range(KD):
                            nc.tensor.matmul(out=psH3[:nlen, :], lhsT=xt[:, k, :nlen],
                                             rhs=w3t[:, k, :], start=(k == 0), stop=(k == KD - 1))
                        hs = ep.tile([P, 512], F32, tag="hs")
                        nc.scalar.activation(hs[:nlen, :], psH1[:nlen, :], AF.Silu)
                        h3 = ep.tile([P, 512], F32, tag="h3")
                        nc.scalar.copy(h3[:nlen, :], psH3[:nlen, :])
                        hrem = ep.tile([P, 512], F32, tag="hrem")
                        nc.vector.tensor_mul(hrem[:nlen, :], hs[:nlen, :], h3[:nlen, :])
                        for m in range(KI):
                            pst = pp2.tile([P, P], F32, tag="pst")
                            nc.tensor.transpose(out=pst[:, :nlen],
                                                in_=hrem[:nlen, m * P:(m + 1) * P],
                                                identity=ident[:nlen, :nlen])
                            nc.vector.tensor_copy(ht[:, m, :nlen], pst[:, :nlen])
                    else:
                        for m in range(KI):
                            up_proj(xt, w1t, w3t, b1t, b3t, ht, m, nlen, KD)
                    for c in range(nch):
                        cl = min(P, nlen - c * P)
                        zt = zp.tile([P, DIM], F16, tag="zt")
                        for h in range(2):
                            psz = pp2.tile([P, 512], F32, tag="psz")
                            for k in range(KI):
                                nc.tensor.matmul(out=psz[:cl, :],
                                                 lhsT=ht[:, k, c * P:c * P + cl],
                                                 rhs=w2t[:, k, h * 512:(h + 1) * 512],
                                                 start=(k == 0), stop=(k == KI - 1))
                            nc.scalar.activation(zt[:cl, h * 512:(h + 1) * 512], psz[:cl, :],
                                                 AF.Copy, scale=pwt[:cl, c:c + 1])
                        nc.gpsimd.indirect_dma_start(
                            out=zbuf.ap(),
                            out_offset=bass.IndirectOffsetOnAxis(ap=sot[:cl, c:c + 1], axis=0),
                            in_=zt[:cl, :],
                            in_offset=None,
                        )

            # ------- phase B+C fused: shared expert + combine per 512 tokens -------
            if not shared_tiles:
                load_shared()
            s1a, s1b = shared_tiles["s1a"], shared_tiles["s1b"]
            s3a, s3b = shared_tiles["s3a"], shared_tiles["s3b"]
            s2a, s2b = shared_tiles["s2a"], shared_tiles["s2b"]
            sb1t = cst.tile([P, KS], F32)
            nc.sync.dma_start(out=sb1t[:], in_=sb1.ap()[0].rearrange("(m p) -> p m", p=P))
            sb3t = cst.tile([P, KS], F32)
            nc.sync.dma_start(out=sb3t[:], in_=sb3.ap()[0].rearrange("(m p) -> p m", p=P))
            b2t = cst.tile([E + 1, DIM], F32R)
            nc.sync.dma_start(out=b2t[:], in_=b2a.ap())

            for (n0, nlen) in _chunks(T, 512):
                xt = xp.tile([P, KD, 512], BF16, tag="xg")
                nc.sync.dma_start(
                    out=xt[:, :, :nlen],
                    in_=xT.ap()[:, n0:n0 + nlen].rearrange("(k p) n -> p k n", p=P),
                )
                ht = hp.tile([P, KS, 512], BF16, tag="ht")
                for m in range(KS):
                    ps1 = pp1.tile([P, 512], F32, tag="ps1")
                    for k in range(KD):
                        w = s1a if k < 4 else s1b
                        nc.tensor.matmul(out=ps1[:, :nlen], lhsT=w[:, k % 4, m * P:(m + 1) * P],
                                         rhs=xt[:, k, :nlen], start=(k == 0), stop=(k == KD - 1))
                    ps3 = pp1.tile([P, 512], F32, tag="ps3")
                    for k in range(KD):
                        w = s3a if k < 4 else s3b
                        nc.tensor.matmul(out=ps3[:, :nlen], lhsT=w[:, k % 4, m * P:(m + 1) * P],
                                         rhs=xt[:, k, :nlen], start=(k == 0), stop=(k == KD - 1))
                    hs = ep.tile([P, 512], F32, tag="hs")
                    nc.scalar.activation(hs[:, :nlen], ps1[:, :nlen], AF.Silu,
                                         bias=sb1t[:, m:m + 1], scale=1.0)
                    h3 = ep.tile([P, 512], F32, tag="h3")
                    nc.scalar.activation(h3[:, :nlen], ps3[:, :nlen], AF.Identity,
                                         bias=sb3t[:, m:m + 1], scale=1.0)
                    nc.vector.tensor_mul(ht[:, m, :nlen], hs[:, :nlen], h3[:, :nlen])
                for c in range(4):
                    t0 = n0 + c * P
                    cwt = cp.tile([E + 1, P], F32R, tag="cwt")
                    nc.sync.dma_start(out=cwt[:], in_=cwT1.ap()[:, t0:t0 + P])
                    yt = cp.tile([P, DIM], F32, tag="yt")
                    for h in range(2):
                        psz = pp2.tile([P, 512], F32, tag="psz")
                        for k in range(KS):
                            w = s2a if k < 4 else s2b
                            nc.tensor.matmul(out=psz[:, :],
                                             lhsT=ht[:, k, c * P:(c + 1) * P],
                                             rhs=w[:, k % 4, h * 512:(h + 1) * 512],
                                             start=(k == 0), stop=False)
                        nc.tensor.matmul(out=psz[:, :], lhsT=cwt[:],
                                         rhs=b2t[:, h * 512:(h + 1) * 512],
                                         start=False, stop=True)
                        nc.scalar.copy(yt[:, h * 512:(h + 1) * 512], psz[:, :])
                    for k in range(4):
                        zt = cp.tile([P, DIM], F16, tag="zc")
                        nc.sync.dma_start(out=zt[:], in_=zbuf.ap()[k * T + t0:k * T + t0 + P, :])
                        nc.vector.tensor_add(yt[:], yt[:], zt[:])
                    nc.sync.dma_start(out=y.ap()[t0:t0 + P, :], in_=yt[:])
    return nc


def _host_route(cw, T):
    """From dense combine weights cw[T, E] build (per-core) routing lists.
    Returns tokens[e] (np arrays), weights[e], slot_of_pair[e]."""
    nz = cw > 0.0
    counts = nz.sum(1)
    toks, wts, slots = [], [], []
    slot_ctr = np.zeros(T, np.int64)
    # tokens with more than TOPK positives (ties): keep top TOPK by value
    drop = {}
    for t in np.nonzero(counts > TOPK)[0]:
        vals = cw[t]
        order = np.argsort(-vals, kind="stable")
        drop[t] = set(order[TOPK:][vals[order[TOPK:]] > 0].tolist())
    for e in range(E):
        tk = np.nonzero(nz[:, e])[0]
        if drop:
            tk = np.array([t for t in tk if not (t in drop and e in drop[t])], dtype=np.int64)
        toks.append(tk)
        wts.append(cw[tk, e])
        sl = slot_ctr[tk].copy()
        slot_ctr[tk] += 1
        slots.append(sl)
    return toks, wts, slots, slot_ctr


def kernel(x, gw, gb, w1, b1, w3, b3, w2, b2, sw1, sb1, sw3, sb3, sw2, sb2):
    x = np.ascontiguousarray(np.asarray(x, np.float32))
    B, S, _ = x.shape
    T = (B * S) // NCORES
    xs = x.reshape(NCORES, T, DIM)
    xT = np.ascontiguousarray(xs.transpose(0, 2, 1))  # [NCORES, DIM, T]
    gw = np.ascontiguousarray(np.asarray(gw, np.float32))
    gb2d = np.asarray(gb, np.float32).reshape(1, E)

    # ---- launch 1: gate ----
    nc1 = build_gate(T)
    nc1.compile()
    in_maps = [{"xT": xT[c], "gw": gw, "gb": gb2d} for c in range(NCORES)]
    res1 = run_bass_kernel_spmd(nc1, in_maps, core_ids=list(range(NCORES)))
    cws = [res1.results[c]["cw"] for c in range(NCORES)]  # [T, E] each

    # ---- host: build routing metadata (data movement only) ----
    routed = [_host_route(cws[c], T) for c in range(NCORES)]
    cnt = np.array([[len(routed[c][0][e]) for e in range(E)] for c in range(NCORES)])
    seg_len = cnt.max(0)  # shared static plan across cores
    seg_len = ((seg_len + 3) // 4) * 4  # fp32r matmuls need an even moving dim
    seg_start = np.concatenate([[0], np.cumsum(seg_len)]).astype(int)
    Lsum = int(seg_len.sum())
    DUMMY = 4 * T

    xTb = xT.astype(BF)  # bf16 copy for the main kernel (gate stays fp32)

    xgs, pws, soffs, cwT1s = [], [], [], []
    for c in range(NCORES):
        toks, wts, slots, slot_ctr = routed[c]
        xg = np.zeros((DIM, Lsum), BF)
        pwv = np.zeros((Lsum, 1), np.float32)
        sov = np.full((Lsum, 1), DUMMY, np.int32)
        pad_list = []
        for e in range(E):
            s0 = seg_start[e]
            n = len(toks[e])
            if n:
                xg[:, s0:s0 + n] = xT[c][:, toks[e]]
                pwv[s0:s0 + n, 0] = wts[e]
                sov[s0:s0 + n, 0] = (slots[e] * T + toks[e]).astype(np.int32)
            pad_list.extend(range(s0 + n, s0 + int(seg_len[e])))
        # route missing (token, slot) pairs (from dropped ties) to padding pairs,
        # which compute exact zeros -> correct "no contribution" rows.
        miss = [(t, s) for t in np.nonzero(slot_ctr < TOPK)[0]
                for s in range(int(slot_ctr[t]), TOPK)]
        assert len(miss) <= len(pad_list), "not enough padding slots"
        for (t, s), j in zip(miss, pad_list):
            sov[j, 0] = np.int32(s * T + t)
        xgs.append(xg)
        pws.append(pwv)
        soffs.append(sov)
        cwT1s.append(np.ascontiguousarray(
            np.concatenate([cws[c].T, np.ones((1, T), np.float32)], 0)))

    b2a = np.ascontiguousarray(np.concatenate(
        [np.asarray(b2, np.float32), np.asarray(sb2, np.float32).reshape(1, DIM)], 0))

    # ---- launch 2: main ----
    # note: the token-stationary remainder path (rem_fast) measured slower on
    # hardware (1997us vs 1892us) - keep it disabled.
    nc2 = build_main(T, seg_len, Lsum, rem_fast=False)
    nc2.compile()
    w1c = np.ascontiguousarray(np.asarray(w1, np.float32))
    w3c = np.ascontiguousarray(np.asarray(w3, np.float32))
    w2c = np.ascontiguousarray(np.asarray(w2, np.float32))
    in_maps = [{
        "xT": xT[c], "xg": xgs[c], "pw": pws[c], "soff": soffs[c],
        "cwT1": cwT1s[c], "b2a": b2a,
        "w1": w1c, "b1": np.asarray(b1, np.float32),
        "w3": w3c, "b3": np.asarray(b3, np.float32),
        "w2": w2c,
        "sw1": np.asarray(sw1, np.float32), "sb1": np.asarray(sb1, np.float32).reshape(1, SINTER),
        "sw3": np.asarray(sw3, np.float32), "sb3": np.asarray(sb3, np.float32).reshape(1, SINTER),
        "sw2": np.asarray(sw2, np.float32),
    } for c in range(NCORES)]
    res2 = run_bass_kernel_spmd(nc2, in_maps, core_ids=list(range(NCORES)))
    ys = np.stack([res2.results[c]["y"] for c in range(NCORES)])
    return ys.reshape(B, S, DIM)



# revision 19
# speedup vs baseline: 1.3506x; 1.3506x over previous
"""MoE (DeepSeek-style gate + 32 routed SwiGLU experts + shared expert) on 8 trn2 cores.

Strategy: data-parallel over tokens (batch dim 8 -> 1 slab per core), expert
weights replicated.  Two device launches per call:

  1. gate kernel: computes dense combine-weights cw[T, E] (softmax + grouped
     top-k routing) on device.
  2. main kernel: per expert, gathers its tokens (host builds the gather
     layout from cw - pure data movement), runs the SwiGLU expert MLP on
     device with fp32r matmuls, scales by routing weight, scatters rows into
     a slot buffer, then combines slots + shared-expert output + (b2/sb2 via
     a small cw @ [b2;sb2] matmul) into y.

All arithmetic happens on device; the host only reshapes/permutes data.
"""

import sys

sys.path.insert(0, "/opt/trn_rl_repo")

import ml_dtypes
import numpy as np

BF = ml_dtypes.bfloat16

import concourse.bacc as bacc
import concourse.mybir as mybir
import concourse.tile as tile
from concourse import bass
from concourse.bass_utils import run_bass_kernel_spmd
from concourse.masks import make_identity

NCORES = 8
DIM = 1024
INTER = 512
E = 32
TOPK = 4
GROUPS = 8
TOPK_G = 4
SINTER = 1024
P = 128
KD = DIM // P     # 8 k-tiles over dim
KI = INTER // P   # 4 k-tiles over inter
KS = SINTER // P  # 8 k-tiles over shared inter

F32 = mybir.dt.float32
F32R = mybir.dt.float32r
F16 = mybir.dt.float16
BF16 = mybir.dt.bfloat16
I32 = mybir.dt.int32
AF = mybir.ActivationFunctionType
OP = mybir.AluOpType
AX = mybir.AxisListType


def _chunks(total, size):
    out = []
    off = 0
    while off < total:
        out.append((off, min(size, total - off)))
        off += size
    return out


def build_gate(T):
    """Gate: logits via [E, tok]-oriented matmuls (512 moving rows, fp32r rate
    1 cyc/row), PE-transpose to [tok, E], then a 4-tile-batched softmax +
    grouped top-k chain on the vector engine."""
    NB = 4           # token tiles batched per vector op
    CH = NB * P      # 512-token chunks
    nc = bacc.Bacc("TRN2", target_bir_lowering=False)
    xT = nc.dram_tensor("xT", [DIM, T], F32, kind="ExternalInput")
    gw = nc.dram_tensor("gw", [DIM, E], F32, kind="ExternalInput")
    gb = nc.dram_tensor("gb", [1, E], F32, kind="ExternalInput")
    ones = nc.dram_tensor("ones", [1, CH], F32, kind="ExternalInput")
    cw = nc.dram_tensor("cw", [T, E], F32, kind="ExternalOutput")
    with tile.TileContext(nc) as tc:
        with tc.tile_pool(name="cst", bufs=1) as cst, \
             tc.tile_pool(name="sb", bufs=3) as sb, \
             tc.tile_pool(name="ps", bufs=3, space="PSUM") as ps, \
             tc.tile_pool(name="ps2", bufs=3, space="PSUM") as ps2:
            gwt = cst.tile([P, KD, E], F32)
            nc.sync.dma_start(out=gwt[:], in_=gw.ap().rearrange("(k p) e -> p k e", p=P))
            gbt = cst.tile([1, E], F32)
            nc.sync.dma_start(out=gbt[:], in_=gb.ap())
            onet = cst.tile([1, CH], F32)
            nc.sync.dma_start(out=onet[:], in_=ones.ap())
            ident = cst.tile([P, P], F32)
            make_identity(nc, ident[:])
            for t in range(T // CH):
                xt = sb.tile([P, KD, CH], F32, tag="xt")
                nc.sync.dma_start(
                    out=xt[:],
                    in_=xT.ap()[:, t * CH:(t + 1) * CH].rearrange("(k p) n -> p k n", p=P),
                )
                # logits [E, CH]: experts stationary, tokens moving (512 rows)
                s = ps.tile([E, CH], F32, tag="s")
                for k in range(KD):
                    nc.tensor.matmul(out=s[:], lhsT=gwt[:, k, :], rhs=xt[:, k, :],
                                     start=(k == 0), stop=False)
                nc.tensor.matmul(out=s[:], lhsT=gbt[:], rhs=onet[:], start=False, stop=True)
                st = sb.tile([E, CH], F32, tag="st")
                nc.scalar.copy(st[:], s[:])
                # transpose to [tok, E], NB tiles side by side
                lg = sb.tile([P, NB, E], F32, tag="lg")
                for c in range(NB):
                    pst = ps2.tile([P, E], F32, tag="pt")
                    nc.tensor.transpose(out=pst[:, :E], in_=st[:, c * P:(c + 1) * P],
                                        identity=ident[:E, :E])
                    nc.vector.tensor_copy(lg[:, c, :], pst[:, :E])
                # batched softmax over the expert axis
                negmax = sb.tile([P, NB], F32, tag="negmax")
                nc.vector.tensor_reduce(out=negmax[:], in_=lg[:], op=OP.max, axis=AX.X,
                                        negate=True)
                sm = sb.tile([P, NB, E], F32, tag="sm")
                nc.vector.tensor_tensor(
                    out=sm[:], in0=lg[:],
                    in1=negmax[:].unsqueeze(2).to_broadcast([P, NB, E]), op=OP.add)
                et = sb.tile([P, NB, E], F32, tag="et")
                nc.scalar.activation(et[:], sm[:], AF.Exp)
                ssum = sb.tile([P, NB], F32, tag="ssum")
                nc.vector.tensor_reduce(out=ssum[:], in_=et[:], op=OP.add, axis=AX.X)
                rsum = sb.tile([P, NB], F32, tag="rsum")
                nc.vector.reciprocal(rsum[:], ssum[:])
                sc = sb.tile([P, NB, E], F32, tag="sc")
                nc.vector.tensor_tensor(
                    out=sc[:], in0=et[:],
                    in1=rsum[:].unsqueeze(2).to_broadcast([P, NB, E]), op=OP.mult)
                # group scores: sum of top-2 scores within each group of 4.
                # top2sum(a,b,c,d) = max(a+b, c+d, max(a,b)+max(c,d))
                g = sc[:].rearrange("p n (g c) -> p n g c", c=4)
                ga = sb.tile([P, NB, GROUPS], F32, tag="ga")
                gbv = sb.tile([P, NB, GROUPS], F32, tag="gbv")
                m1 = sb.tile([P, NB, GROUPS], F32, tag="m1")
                gsc = sb.tile([P, NB, GROUPS], F32, tag="gsc")
                nc.vector.tensor_add(ga[:], g[:, :, :, 0], g[:, :, :, 1])
                nc.vector.tensor_add(gbv[:], g[:, :, :, 2], g[:, :, :, 3])
                nc.vector.tensor_tensor(out=m1[:], in0=g[:, :, :, 0], in1=g[:, :, :, 1], op=OP.max)
                nc.vector.tensor_tensor(out=gsc[:], in0=g[:, :, :, 2], in1=g[:, :, :, 3], op=OP.max)
                nc.vector.tensor_add(m1[:], m1[:], gsc[:])
                nc.vector.tensor_tensor(out=ga[:], in0=ga[:], in1=gbv[:], op=OP.max)
                nc.vector.tensor_tensor(out=gsc[:], in0=ga[:], in1=m1[:], op=OP.max)
                # keep the top-4 groups (MAX8 is whole-free-axis: per tile)
                srt = sb.tile([P, NB, 8], F32, tag="srt")
                for n in range(NB):
                    nc.vector.max(srt[:, n, :], gsc[:, n, :])
                keep = sb.tile([P, NB, GROUPS], F32, tag="keep")
                nc.vector.tensor_tensor(
                    out=keep[:], in0=gsc[:],
                    in1=srt[:, :, 3:4].to_broadcast([P, NB, GROUPS]), op=OP.is_ge)
                # mask scores to kept groups, take top-4 experts
                masked = sb.tile([P, NB, E], F32, tag="masked")
                nc.vector.tensor_tensor(
                    out=masked[:].rearrange("p n (g c) -> p n g c", c=4),
                    in0=g,
                    in1=keep[:].unsqueeze(3).to_broadcast([P, NB, GROUPS, 4]),
                    op=OP.mult,
                )
                srt2 = sb.tile([P, NB, 8], F32, tag="srt2")
                for n in range(NB):
                    nc.vector.max(srt2[:, n, :], masked[:, n, :])
                sel = sb.tile([P, NB, E], F32, tag="sel")
                nc.vector.tensor_tensor(
                    out=sel[:], in0=masked[:],
                    in1=srt2[:, :, 3:4].to_broadcast([P, NB, E]), op=OP.is_ge)
                cwt = sb.tile([P, NB, E], F32, tag="cwt")
                nc.vector.tensor_mul(cwt[:], sel[:], masked[:])
                nc.sync.dma_start(
                    out=cw.ap()[t * CH:(t + 1) * CH, :].rearrange("(n p) e -> p n e", p=P),
                    in_=cwt[:])
    return nc


def build_main(T, seg_len, Lsum, rem_fast=False):
    """seg_len[e]: padded token count for expert e (same across cores).
    Lsum = sum(seg_len).  zbuf rows: slot k of token t at k*T+t, dummy
    scratch rows (padding pairs) at 4*T."""
    nc = bacc.Bacc("TRN2", target_bir_lowering=False)
    xT = nc.dram_tensor("xT", [DIM, T], BF16, kind="ExternalInput")
    xg = nc.dram_tensor("xg", [DIM, Lsum], BF16, kind="ExternalInput")
    pw = nc.dram_tensor("pw", [Lsum, 1], F32, kind="ExternalInput")
    soff = nc.dram_tensor("soff", [Lsum, 1], I32, kind="ExternalInput")
    cwT1 = nc.dram_tensor("cwT1", [E + 1, T], F32R, kind="ExternalInput")
    b2a = nc.dram_tensor("b2a", [E + 1, DIM], F32R, kind="ExternalInput")
    w1 = nc.dram_tensor("w1", [E, DIM, INTER], BF16, kind="ExternalInput")
    b1 = nc.dram_tensor("b1", [E, INTER], F32, kind="ExternalInput")
    w3 = nc.dram_tensor("w3", [E, DIM, INTER], BF16, kind="ExternalInput")
    b3 = nc.dram_tensor("b3", [E, INTER], F32, kind="ExternalInput")
    w2 = nc.dram_tensor("w2", [E, INTER, DIM], BF16, kind="ExternalInput")
    sw1 = nc.dram_tensor("sw1", [DIM, SINTER], BF16, kind="ExternalInput")
    sb1 = nc.dram_tensor("sb1", [1, SINTER], F32, kind="ExternalInput")
    sw3 = nc.dram_tensor("sw3", [DIM, SINTER], BF16, kind="ExternalInput")
    sb3 = nc.dram_tensor("sb3", [1, SINTER], F32, kind="ExternalInput")
    sw2 = nc.dram_tensor("sw2", [SINTER, DIM], BF16, kind="ExternalInput")
    y = nc.dram_tensor("y", [T, DIM], F32, kind="ExternalOutput")
    zbuf = nc.dram_tensor("zbuf", [4 * T + P, DIM], F16)

    seg_start = np.concatenate([[0], np.cumsum(seg_len)]).astype(int)

    from contextlib import ExitStack
    with tile.TileContext(nc) as tc:
        with ExitStack() as ctx:
            cst = ctx.enter_context(tc.tile_pool(name="cst", bufs=1))
            wp = ctx.enter_context(tc.tile_pool(name="wp", bufs=2))
            xp = ctx.enter_context(tc.tile_pool(name="xp", bufs=2))
            hp = ctx.enter_context(tc.tile_pool(name="hp", bufs=2))
            ep = ctx.enter_context(tc.tile_pool(name="ep", bufs=2))
            zp = ctx.enter_context(tc.tile_pool(name="zp", bufs=2))
            cp = ctx.enter_context(tc.tile_pool(name="cp", bufs=3))
            pp1 = ctx.enter_context(tc.tile_pool(name="pp1", bufs=2, space="PSUM"))
            pp2 = ctx.enter_context(tc.tile_pool(name="pp2", bufs=4, space="PSUM"))

            ident = cst.tile([P, P], F32)
            make_identity(nc, ident[:])

            def up_proj(xt, w1t, w3t, b1t, b3t, ht, m, nlen, kk):
                """ht[:, m, :nlen] = silu(w1^T x + b1) * (w3^T x + b3) for inter tile m."""
                ps1 = pp1.tile([P, 512], F32, tag="ps1")
                for k in range(kk):
                    nc.tensor.matmul(out=ps1[:, :nlen], lhsT=w1t[:, k, m * P:(m + 1) * P],
                                     rhs=xt[:, k, :nlen], start=(k == 0), stop=(k == kk - 1))
                ps3 = pp1.tile([P, 512], F32, tag="ps3")
                for k in range(kk):
                    nc.tensor.matmul(out=ps3[:, :nlen], lhsT=w3t[:, k, m * P:(m + 1) * P],
                                     rhs=xt[:, k, :nlen], start=(k == 0), stop=(k == kk - 1))
                hs = ep.tile([P, 512], F32, tag="hs")
                nc.scalar.activation(hs[:, :nlen], ps1[:, :nlen], AF.Silu,
                                     bias=b1t[:, m:m + 1], scale=1.0)
                h3 = ep.tile([P, 512], F32, tag="h3")
                nc.scalar.activation(h3[:, :nlen], ps3[:, :nlen], AF.Identity,
                                     bias=b3t[:, m:m + 1], scale=1.0)
                nc.vector.tensor_mul(ht[:, m, :nlen], hs[:, :nlen], h3[:, :nlen])

            # ---------------- phase A: routed experts ----------------
            order = [e for e in range(E) if seg_len[e] > 0]
            shared_tiles = {}

            def load_shared():
                s1a = wp.tile([P, 4, SINTER], BF16, tag="w1e")
                s1b = wp.tile([P, 4, SINTER], BF16, tag="w1e")
                nc.sync.dma_start(out=s1a[:], in_=sw1.ap()[:512, :].rearrange("(k p) i -> p k i", p=P))
                nc.sync.dma_start(out=s1b[:], in_=sw1.ap()[512:, :].rearrange("(k p) i -> p k i", p=P))
                s3a = wp.tile([P, 4, SINTER], BF16, tag="w3e")
                s3b = wp.tile([P, 4, SINTER], BF16, tag="w3e")
                nc.sync.dma_start(out=s3a[:], in_=sw3.ap()[:512, :].rearrange("(k p) i -> p k i", p=P))
                nc.sync.dma_start(out=s3b[:], in_=sw3.ap()[512:, :].rearrange("(k p) i -> p k i", p=P))
                s2a = wp.tile([P, 4, DIM], BF16, tag="w2e")
                s2b = wp.tile([P, 4, DIM], BF16, tag="w2e")
                nc.sync.dma_start(out=s2a[:], in_=sw2.ap()[:512, :].rearrange("(k p) d -> p k d", p=P))
                nc.sync.dma_start(out=s2b[:], in_=sw2.ap()[512:, :].rearrange("(k p) d -> p k d", p=P))
                shared_tiles.update(s1a=s1a, s1b=s1b, s3a=s3a, s3b=s3b, s2a=s2a, s2b=s2b)

            for ei, e in enumerate(order):
                e = int(e)
                w1t = wp.tile([P, KD, INTER], BF16, tag="w1e")
                nc.sync.dma_start(out=w1t[:], in_=w1.ap()[e].rearrange("(k p) i -> p k i", p=P))
                w3t = wp.tile([P, KD, INTER], BF16, tag="w3e")
                nc.sync.dma_start(out=w3t[:], in_=w3.ap()[e].rearrange("(k p) i -> p k i", p=P))
                w2t = wp.tile([P, KI, DIM], BF16, tag="w2e")
                nc.sync.dma_start(out=w2t[:], in_=w2.ap()[e].rearrange("(k p) d -> p k d", p=P))
                b1t = wp.tile([P, KI], F32, tag="b1e")
                nc.sync.dma_start(out=b1t[:], in_=b1.ap()[e].rearrange("(m p) -> p m", p=P))
                b3t = wp.tile([P, KI], F32, tag="b3e")
                nc.sync.dma_start(out=b3t[:], in_=b3.ap()[e].rearrange("(m p) -> p m", p=P))
                for (n0, nlen) in _chunks(int(seg_len[e]), 512):
                    g0 = seg_start[e] + n0
                    xt = xp.tile([P, KD, 512], BF16, tag="xg")
                    nc.sync.dma_start(
                        out=xt[:, :, :nlen],
                        in_=xg.ap()[:, g0:g0 + nlen].rearrange("(k p) n -> p k n", p=P),
                    )
                    nch = (nlen + P - 1) // P
                    pwt = ep.tile([P, 4], F32, tag="pwt")
                    sot = ep.tile([P, 4], I32, tag="sot")
                    for c in range(nch):
                        cl = min(P, nlen - c * P)
                        nc.sync.dma_start(out=pwt[:cl, c:c + 1], in_=pw.ap()[g0 + c * P:g0 + c * P + cl, :])
                        nc.sync.dma_start(out=sot[:cl, c:c + 1], in_=soff.ap()[g0 + c * P:g0 + c * P + cl, :])
                    ht = hp.tile([P, KS, 512], BF16, tag="ht")
                    if rem_fast and nlen <= P:
                        # token-stationary up-proj (8+8 matmuls instead of 64),
                        # then PE-transpose H -> HT.  Valid because b1/b3 == 0.
                        psH1 = pp1.tile([P, 512], F32, tag="ps1")
                        for k in range(KD):
                            nc.tensor.matmul(out=psH1[:nlen, :], lhsT=xt[:, k, :nlen],
                                             rhs=w1t[:, k, :], start=(k == 0), stop=(k == KD - 1))
                        psH3 = pp1.tile([P, 512], F32, tag="ps3")
                        for k in range(KD):
                            nc.tensor.matmul(out=psH3[:nlen, :], lhsT=xt[:, k, :nlen],
                                             rhs=w3t[:, k, :], start=(k == 0), stop=(k == KD - 1))
                        hs = ep.tile([P, 512], F32, tag="hs")
                        nc.scalar.activation(hs[:nlen, :], psH1[:nlen, :], AF.Silu)
                        h3 = ep.tile([P, 512], F32, tag="h3")
                        nc.scalar.copy(h3[:nlen, :], psH3[:nlen, :])
                        hrem = ep.tile([P, 512], F32, tag="hrem")
                        nc.vector.tensor_mul(hrem[:nlen, :], hs[:nlen, :], h3[:nlen, :])
                        for m in range(KI):
                            pst = pp2.tile([P, P], F32, tag="pst")
                            nc.tensor.transpose(out=pst[:, :nlen],
                                                in_=hrem[:nlen, m * P:(m + 1) * P],
                                                identity=ident[:nlen, :nlen])
                            nc.vector.tensor_copy(ht[:, m, :nlen], pst[:, :nlen])
                    else:
                        for m in range(KI):
                            up_proj(xt, w1t, w3t, b1t, b3t, ht, m, nlen, KD)
                    for c in range(nch):
                        cl = min(P, nlen - c * P)
                        zt = zp.tile([P, DIM], F16, tag="zt")
                        for h in range(2):
                            psz = pp2.tile([P, 512], F32, tag="psz")
                            for k in range(KI):
                                nc.tensor.matmul(out=psz[:cl, :],
                                                 lhsT=ht[:, k, c * P:c * P + cl],
                                                 rhs=w2t[:, k, h * 512:(h + 1) * 512],
                                                 start=(k == 0), stop=(k == KI - 1))
                            nc.scalar.activation(zt[:cl, h * 512:(h + 1) * 512], psz[:cl, :],
                                                 AF.Copy, scale=pwt[:cl, c:c + 1])
                        nc.gpsimd.indirect_dma_start(
                            out=zbuf.ap(),
                            out_offset=bass.IndirectOffsetOnAxis(ap=sot[:cl, c:c + 1], axis=0),
                            in_=zt[:cl, :],
                            in_offset=None,
                        )

            # ------- phase B+C fused: shared expert + combine per 512 tokens -------
            if not shared_tiles:
                load_shared()
            s1a, s1b = shared_tiles["s1a"], shared_tiles["s1b"]
            s3a, s3b = shared_tiles["s3a"], shared_tiles["s3b"]
            s2a, s2b = shared_tiles["s2a"], shared_tiles["s2b"]
            sb1t = cst.tile([P, KS], F32)
            nc.sync.dma_start(out=sb1t[:], in_=sb1.ap()[0].rearrange("(m p) -> p m", p=P))
            sb3t = cst.tile([P, KS], F32)
            nc.sync.dma_start(out=sb3t[:], in_=sb3.ap()[0].rearrange("(m p) -> p m", p=P))
            b2t = cst.tile([E + 1, DIM], F32R)
            nc.sync.dma_start(out=b2t[:], in_=b2a.ap())

            for (n0, nlen) in _chunks(T, 512):
                xt = xp.tile([P, KD, 512], BF16, tag="xg")
                nc.sync.dma_start(
                    out=xt[:, :, :nlen],
                    in_=xT.ap()[:, n0:n0 + nlen].rearrange("(k p) n -> p k n", p=P),
                )
                ht = hp.tile([P, KS, 512], BF16, tag="ht")
                for m in range(KS):
                    ps1 = pp1.tile([P, 512], F32, tag="ps1")
                    for k in range(KD):
                        w = s1a if k < 4 else s1b
                        nc.tensor.matmul(out=ps1[:, :nlen], lhsT=w[:, k % 4, m * P:(m + 1) * P],
                                         rhs=xt[:, k, :nlen], start=(k == 0), stop=(k == KD - 1))
                    ps3 = pp1.tile([P, 512], F32, tag="ps3")
                    for k in range(KD):
                        w = s3a if k < 4 else s3b
                        nc.tensor.matmul(out=ps3[:, :nlen], lhsT=w[:, k % 4, m * P:(m + 1) * P],
                                         rhs=xt[:, k, :nlen], start=(k == 0), stop=(k == KD - 1))
                    hs = ep.tile([P, 512], F32, tag="hs")
                    nc.scalar.activation(hs[:, :nlen], ps1[:, :nlen], AF.Silu,
                                         bias=sb1t[:, m:m + 1], scale=1.0)
                    h3 = ep.tile([P, 512], F32, tag="h3")
                    nc.scalar.activation(h3[:, :nlen], ps3[:, :nlen], AF.Identity,
                                         bias=sb3t[:, m:m + 1], scale=1.0)
                    nc.vector.tensor_mul(ht[:, m, :nlen], hs[:, :nlen], h3[:, :nlen])
                for c in range(4):
                    t0 = n0 + c * P
                    cwt = cp.tile([E + 1, P], F32R, tag="cwt")
                    nc.sync.dma_start(out=cwt[:], in_=cwT1.ap()[:, t0:t0 + P])
                    yt = cp.tile([P, DIM], F32, tag="yt")
                    for h in range(2):
                        psz = pp2.tile([P, 512], F32, tag="psz")
                        for k in range(KS):
                            w = s2a if k < 4 else s2b
                            nc.tensor.matmul(out=psz[:, :],
                                             lhsT=ht[:, k, c * P:(c + 1) * P],
                                             rhs=w[:, k % 4, h * 512:(h + 1) * 512],
                                             start=(k == 0), stop=False)
                        nc.tensor.matmul(out=psz[:, :], lhsT=cwt[:],
                                         rhs=b2t[:, h * 512:(h + 1) * 512],
                                         start=False, stop=True)
                        nc.scalar.copy(yt[:, h * 512:(h + 1) * 512], psz[:, :])
                    for k in range(4):
                        zt = cp.tile([P, DIM], F16, tag="zc")
                        nc.sync.dma_start(out=zt[:], in_=zbuf.ap()[k * T + t0:k * T + t0 + P, :])
                        nc.vector.tensor_add(yt[:], yt[:], zt[:])
                    nc.sync.dma_start(out=y.ap()[t0:t0 + P, :], in_=yt[:])
    return nc


def _host_route(cw, T):
    """From dense combine weights cw[T, E] build (per-core) routing lists.
    Returns tokens[e] (np arrays), weights[e], slot_of_pair[e]."""
    nz = cw > 0.0
    counts = nz.sum(1)
    toks, wts, slots = [], [], []
    slot_ctr = np.zeros(T, np.int64)
    # tokens with more than TOPK positives (ties): keep top TOPK by value
    drop = {}
    for t in np.nonzero(counts > TOPK)[0]:
        vals = cw[t]
        order = np.argsort(-vals, kind="stable")
        drop[t] = set(order[TOPK:][vals[order[TOPK:]] > 0].tolist())
    for e in range(E):
        tk = np.nonzero(nz[:, e])[0]
        if drop:
            tk = np.array([t for t in tk if not (t in drop and e in drop[t])], dtype=np.int64)
        toks.append(tk)
        wts.append(cw[tk, e])
        sl = slot_ctr[tk].copy()
        slot_ctr[tk] += 1
        slots.append(sl)
    return toks, wts, slots, slot_ctr


def _balance_tokens(cw_all, T):
    """Assign tokens to cores (T each) so per-(core, expert) counts stay as
    even as possible -> shorter shared seg_len plan.  Data movement only."""
    Tall, Ecnt = cw_all.shape
    nz = cw_all > 0.0
    counts = np.zeros((NCORES, Ecnt), np.int32)
    totals = np.zeros(NCORES, np.int32)
    assign = [[] for _ in range(NCORES)]
    # process tokens grouped by expert signature for stable cache behavior
    for t in range(Tall):
        es = np.nonzero(nz[t])[0]
        best, bestcost = -1, None
        for c in range(NCORES):
            if totals[c] >= T:
                continue
            cost = int(counts[c, es].max()) * 1000 + int(counts[c, es].sum())
            if bestcost is None or cost < bestcost:
                best, bestcost = c, cost
        counts[best, es] += 1
        totals[best] += 1
        assign[best].append(t)
    return np.array([np.array(a, np.int64) for a in assign])


def kernel(x, gw, gb, w1, b1, w3, b3, w2, b2, sw1, sb1, sw3, sb3, sw2, sb2):
    x = np.ascontiguousarray(np.asarray(x, np.float32))
    B, S, _ = x.shape
    Tall = B * S
    T = Tall // NCORES
    xf = x.reshape(Tall, DIM)
    xs = xf.reshape(NCORES, T, DIM)
    xT = np.ascontiguousarray(xs.transpose(0, 2, 1))  # [NCORES, DIM, T]
    gw = np.ascontiguousarray(np.asarray(gw, np.float32))
    gb2d = np.asarray(gb, np.float32).reshape(1, E)

    # ---- launch 1: gate (slab split; cw[t] depends only on x[t]) ----
    nc1 = build_gate(T)
    nc1.compile()
    ones512 = np.ones((1, 512), np.float32)
    in_maps = [{"xT": xT[c], "gw": gw, "gb": gb2d, "ones": ones512} for c in range(NCORES)]
    res1 = run_bass_kernel_spmd(nc1, in_maps, core_ids=list(range(NCORES)))
    cw_all = np.concatenate([res1.results[c]["cw"] for c in range(NCORES)], 0)

    # ---- host: balanced token->core assignment + routing metadata ----
    assign = _balance_tokens(cw_all, T)  # [NCORES, T] token ids
    cws = [cw_all[assign[c]] for c in range(NCORES)]
    routed = [_host_route(cws[c], T) for c in range(NCORES)]
    cnt = np.array([[len(routed[c][0][e]) for e in range(E)] for c in range(NCORES)])
    seg_len = cnt.max(0)  # shared static plan across cores
    seg_len = ((seg_len + 3) // 4) * 4  # fp32r matmuls need an even moving dim
    seg_start = np.concatenate([[0], np.cumsum(seg_len)]).astype(int)
    Lsum = int(seg_len.sum())
    DUMMY = 4 * T

    xfb = xf.astype(BF)  # bf16 tokens for the main kernel (gate stays fp32)
    xTmb = [np.ascontiguousarray(xfb[assign[c]].T) for c in range(NCORES)]

    xgs, pws, soffs, cwT1s = [], [], [], []
    for c in range(NCORES):
        toks, wts, slots, slot_ctr = routed[c]
        xg = np.zeros((DIM, Lsum), BF)
        pwv = np.zeros((Lsum, 1), np.float32)
        sov = np.full((Lsum, 1), DUMMY, np.int32)
        pad_list = []
        for e in range(E):
            s0 = seg_start[e]
            n = len(toks[e])
            if n:
                xg[:, s0:s0 + n] = xTmb[c][:, toks[e]]
                pwv[s0:s0 + n, 0] = wts[e]
                sov[s0:s0 + n, 0] = (slots[e] * T + toks[e]).astype(np.int32)
            pad_list.extend(range(s0 + n, s0 + int(seg_len[e])))
        # route missing (token, slot) pairs (from dropped ties) to padding pairs,
        # which compute exact zeros -> correct "no contribution" rows.
        miss = [(t, s) for t in np.nonzero(slot_ctr < TOPK)[0]
                for s in range(int(slot_ctr[t]), TOPK)]
        assert len(miss) <= len(pad_list), "not enough padding slots"
        for (t, s), j in zip(miss, pad_list):
            sov[j, 0] = np.int32(s * T + t)
        xgs.append(xg)
        pws.append(pwv)
        soffs.append(sov)
        cwT1s.append(np.ascontiguousarray(
            np.concatenate([cws[c].T, np.ones((1, T), np.float32)], 0)))

    b2a = np.ascontiguousarray(np.concatenate(
        [np.asarray(b2, np.float32), np.asarray(sb2, np.float32).reshape(1, DIM)], 0))

    # ---- launch 2: main ----
    nc2 = build_main(T, seg_len, Lsum, rem_fast=False)
    nc2.compile()
    w1c = np.ascontiguousarray(np.asarray(w1, BF))
    w3c = np.ascontiguousarray(np.asarray(w3, BF))
    w2c = np.ascontiguousarray(np.asarray(w2, BF))
    in_maps = [{
        "xT": xTmb[c], "xg": xgs[c], "pw": pws[c], "soff": soffs[c],
        "cwT1": cwT1s[c], "b2a": b2a,
        "w1": w1c, "b1": np.asarray(b1, np.float32),
        "w3": w3c, "b3": np.asarray(b3, np.float32),
        "w2": w2c,
        "sw1": np.asarray(sw1, BF), "sb1": np.asarray(sb1, np.float32).reshape(1, SINTER),
        "sw3": np.asarray(sw3, BF), "sb3": np.asarray(sb3, np.float32).reshape(1, SINTER),
        "sw2": np.asarray(sw2, BF),
    } for c in range(NCORES)]
    res2 = run_bass_kernel_spmd(nc2, in_maps, core_ids=list(range(NCORES)))
    out = np.empty((Tall, DIM), np.float32)
    for c in range(NCORES):
        out[assign[c]] = res2.results[c]["y"]
    return out.reshape(B, S, DIM)

